# revision 14
# baseline (speedup 1.0000x reference)
"""Trainium2 Bass kernel for nn_RelPosRFFBias — factorized Fourier, v5 (full host mirror).

Math: per head h, bias(t,s) = g_h(|c_t - c_s|) with g_h fit as a ~125-tone
cosine+sine series.  In sorted-center order, for t >= s (lower triangle):

  g(d) = sum_k a_k cos(w_k d) + b_k sin(w_k d)
       = sum_k cos_t (a_k cos_s - b_k sin_s) + sin_t (a_k sin_s + b_k cos_s)

so ONE rank-256 matmul per (row-tile, head) with lhs = raw interleaved
cos/sin table U and rhs = rcomb = aq (.) U + bq (.) V (V = pair-swapped U)
evaluates the whole lower block-triangle, diagonal tiles included.  The host
mirrors the strict upper triangle (inter- and intra-tile) by symmetry and
undoes the sort permutation.

v5 vs v4 baseline:
 - no separate P matmuls / diag sign fixups: PE work halves (43us -> ~18us).
 - builds are 2 DVE passes per (head, chunk): tensor_scalar + fused
   scalar_tensor_tensor, both 4x-mode eligible.
 - PSUM evacuation split across Scalar (i=3, i=1) and GpSimd (i=2, i=0).
 - warmup trimmed to ~7 matmuls (p-state ramp needs ~3us, not 6.2us).
 - head-group-column-major schedule: output DMA per (row-tile, 4-head group)
   spread across SP/Act/DVE queues; short tail.
"""

import math

import numpy as np

B, T = 8, 512
RFF, NH = 16, 16
F_MIN, F_MAX = 2.0, 64.0
TWO_PI = 2.0 * math.pi

N_CORES = 8
L_PER = 1.0625
KU = 124
NQMAX = 128
TILE = 128
NT = T // TILE
FIT_LAM = 1e-5
FIT_ITERS = 14
NWARM = 7

_MODULE = None
_LAST_RESULTS = None
_FIT_CACHE = {}


# ---------------------------------------------------------------- host: fit
def _gelu64(x):
    try:
        from scipy.special import erf
    except ImportError:
        erf = np.vectorize(math.erf)
    return 0.5 * x * (1.0 + erf(x / math.sqrt(2.0)))


def _g_of_D(D, phase, W1, b1, W2, b2, freqs):
    arg = TWO_PI * D[:, None] * freqs[None, :] + phase[None, :]
    feats = np.concatenate([np.sin(arg), np.cos(arg)], axis=-1)
    return _gelu64(feats @ W1 + b1) @ W2 + b2


def _tone_grid():
    freqs = np.logspace(math.log10(F_MIN), math.log10(F_MAX), RFF).astype(np.float64)
    uni = np.arange(KU) / L_PER
    cut = uni[-1]
    cand = sorted(set(
        round(f, 6)
        for f in np.concatenate([(freqs[:, None] + freqs[None, :]).ravel(), 2 * freqs])
        if cut + 0.2 < f < 145.0
    ))
    omQ = np.concatenate([uni, np.asarray(cand[: NQMAX - KU], dtype=np.float64)])
    omQ.sort()
    omP = omQ[1:min(len(omQ), 129)]
    return freqs, omQ, omP


def _fit_coefs(phase, W1, b1, W2, b2):
    freqs, omQ, omP = _tone_grid()
    NG = 32768
    Dg = (np.arange(NG) + 0.5) / NG
    G = _g_of_D(Dg, phase, W1, b1, W2, b2, freqs)
    Phi = np.concatenate(
        [np.cos(Dg[:, None] * TWO_PI * omQ[None, :]),
         np.sin(Dg[:, None] * TWO_PI * omP[None, :])], axis=1)
    lam = FIT_LAM * NG
    w = np.ones(NG)
    best = None
    for _ in range(FIT_ITERS):
        Pw = Phi * w[:, None]
        A = Pw.T @ Phi
        A[np.diag_indices_from(A)] += lam
        coef = np.linalg.solve(A, Pw.T @ G)
        res = np.abs(Phi @ coef - G).max(axis=1)
        mx = res.max()
        if best is None or mx < best[0]:
            best = (mx, coef.copy())
        w = w * (0.05 + res / mx)
        w = np.maximum(w / w.mean(), 1e-6)
    mx, coef = best
    return omQ, omP, coef[: len(omQ)], coef[len(omQ):], mx


# ---------------------------------------------------------------- device
def _build_module():
    import concourse.tile as tile
    from concourse import bacc, mybir
    from contextlib import ExitStack

    f32 = mybir.dt.float32
    bf16 = mybir.dt.bfloat16
    Alu = mybir.AluOpType
    Act = mybir.ActivationFunctionType

    nc = bacc.Bacc("TRN2", target_bir_lowering=False, debug=False)

    # packU: raw interleaved cos/sin table, chunk-major [U0 | U1]
    packU_d = nc.dram_tensor("packU", [TILE, 2 * T], bf16, kind="ExternalInput")
    # packW: per (h, c) 2x2-block-diagonal rotation/scale lhsT [128,128],
    # col block at (2h+c)*128.  rcomb_{h,c} = packW_{h,c}^T @ U_c on the PE.
    packW_d = nc.dram_tensor("packW", [TILE, 2 * NH * TILE], bf16,
                             kind="ExternalInput")
    # compact block-lower-triangle outputs, head-major per row-tile i
    out_ds = [nc.dram_tensor(f"out{i}", [TILE, NH * (i + 1) * TILE], bf16,
                             kind="ExternalOutput") for i in range(NT)]

    with tile.TileContext(nc) as tc:
        with ExitStack() as ctx:
            const = ctx.enter_context(tc.tile_pool(name="const", bufs=1))
            rhspool = ctx.enter_context(tc.tile_pool(name="rhs", bufs=1))
            stpool = ctx.enter_context(tc.tile_pool(name="stage", bufs=1))
            # PSUM: pb = [128,1024] (2 banks) x2 bufs for i3/i2 head pairs;
            # pv = [128,1024] (2 banks) x1 buf for the two i1 pairs;
            # pr = [128,512] x2 bufs for rcomb builds + i0 + warmup.
            # 4 + 2 + 2 = 8 banks.
            pb = ctx.enter_context(tc.tile_pool(name="pb", bufs=2, space="PSUM"))
            pv = ctx.enter_context(tc.tile_pool(name="pv", bufs=1, space="PSUM"))
            pr = ctx.enter_context(tc.tile_pool(name="pr", bufs=2, space="PSUM"))

            packW = const.tile([TILE, 2 * NH * TILE], bf16, tag="packW")
            nc.sync.dma_start(packW[:], packW_d.ap())
            packU = const.tile([TILE, 2 * T], bf16, tag="packU")
            nc.scalar.dma_start(packU[:], packU_d.ap())

            # PE warm-up: ~3us of dummy matmuls to finish the p-state ramp
            # while the input DMAs land.
            warm_sb = const.tile([TILE, T], bf16, tag="warm")
            nc.vector.memset(warm_sb[:], 0)
            for _ in range(NWARM):
                warm_ps = pr.tile([TILE, T], f32, tag="pr")
                nc.tensor.matmul(warm_ps[:], warm_sb[:, 0:TILE], warm_sb[:],
                                 start=True, stop=True)

            # rcomb: [128, NH*1024], per head h chunk c at h*1024 + c*512,
            # built on the PE: rcomb_{h,c} = packW_{h,c}^T @ U_c, then
            # evacuated PSUM->SBUF by DVE (mostly) / Scalar.
            rcomb = rhspool.tile([TILE, NH * 2 * T], bf16, tag="rcomb",
                                 name="rcomb")

            def build(h, last_g=False):
                for c in range(2):
                    col = 2 * h + c
                    ps = pr.tile([TILE, T], f32, tag="pr")
                    nc.tensor.matmul(
                        ps[:], packW[:, col * TILE:(col + 1) * TILE],
                        packU[:, c * T:(c + 1) * T], start=True, stop=True)
                    dst = rcomb[:, h * 2 * T + c * T: h * 2 * T + (c + 1) * T]
                    if col % 10 == 1:
                        nc.scalar.activation(dst, ps[:], Act.Identity)
                    else:
                        nc.vector.tensor_copy(dst, ps[:])

            for h in range(6):
                build(h)

            stages = [stpool.tile([TILE, NH * (i + 1) * TILE], bf16,
                                  tag=f"stage{i}", name=f"stage{i}")
                      for i in range(NT)]
            rc_v = rcomb[:].rearrange("p (h x) -> p h x", h=NH)

            def rslice(h, c, n):
                return rcomb[:, h * 2 * T + c * T: h * 2 * T + c * T + n]

            def evac(dst, src, g, k):
                # last head group: split evacuation across both engines to
                # shorten the tail (DVE is otherwise idle by then)
                if g == 3 and k % 2 == 0:
                    nc.vector.tensor_copy(dst, src)
                else:
                    nc.scalar.activation(dst, src, Act.Identity)

            W2_ = 3 * TILE
            W1_ = 2 * TILE
            for g in range(4):
                h0 = 4 * g
                nxt = list(range(h0 + 6, min(h0 + 10, NH))) if g < 3 else []
                # i = 3: head pairs, 2 banks per psum tile, N=512 each
                for m, hp in enumerate((h0, h0 + 2)):
                    ps = pb.tile([TILE, 2 * T], f32, tag="pb")
                    for j in range(2):
                        for c in range(2):
                            nc.tensor.matmul(
                                ps[:, j * T:(j + 1) * T],
                                packU[:, c * T + 3 * TILE: c * T + 4 * TILE],
                                rslice(hp + j, c, T),
                                start=(c == 0), stop=(c == 1),
                                skip_group_check=True)
                    if m == 0 and nxt:
                        build(nxt[0])
                    evac(stages[3][:, hp * T:(hp + 2) * T], ps[:], g, m)
                if len(nxt) > 1:
                    build(nxt[1])
                # i = 1 (two heads per bank) + i = 0 (four heads in one bank)
                psv = pv.tile([TILE, 2 * T], f32, tag="pv")
                for k, hp in enumerate((h0, h0 + 2)):
                    for c in range(2):
                        nc.tensor.matmul(
                            psv[:, k * T:(k + 1) * T],
                            packU[:, c * T + TILE: c * T + 2 * TILE],
                            rc_v[:, hp:hp + 2, c * T: c * T + W1_],
                            start=(c == 0), stop=(c == 1),
                            skip_group_check=True)
                ps0 = pr.tile([TILE, T], f32, tag="pr")
                for c in range(2):
                    nc.tensor.matmul(
                        ps0[:],
                        packU[:, c * T: c * T + TILE],
                        rc_v[:, h0:h0 + 4, c * T: c * T + TILE],
                        start=(c == 0), stop=(c == 1))
                if len(nxt) > 2:
                    build(nxt[2])
                evac(stages[1][:, h0 * W1_:(h0 + 4) * W1_], psv[:], g, 1)
                evac(stages[0][:, h0 * TILE:(h0 + 4) * TILE], ps0[:], g, 0)
                nc.sync.dma_start(
                    out_ds[3].ap()[:, h0 * T:(h0 + 4) * T],
                    stages[3][:, h0 * T:(h0 + 4) * T])
                # i = 2: head pairs, N=384 into each bank of a 2-bank tile
                for m, hp in enumerate((h0, h0 + 2)):
                    ps = pb.tile([TILE, 2 * T], f32, tag="pb")
                    for j in range(2):
                        for c in range(2):
                            nc.tensor.matmul(
                                ps[:, j * T:j * T + W2_],
                                packU[:, c * T + 2 * TILE: c * T + 3 * TILE],
                                rslice(hp + j, c, W2_),
                                start=(c == 0), stop=(c == 1),
                                skip_group_check=True)
                    if m == 0 and len(nxt) > 3:
                        build(nxt[3])
                    evac(stages[2][:, hp * W2_:(hp + 2) * W2_]
                         .rearrange("p (b s) -> p b s", b=2),
                         ps[:].rearrange("p (b s) -> p b s", b=2)[:, :, 0:W2_],
                         g, m)
                nc.sync.dma_start(
                    out_ds[1].ap()[:, h0 * W1_:(h0 + 4) * W1_],
                    stages[1][:, h0 * W1_:(h0 + 4) * W1_])
                nc.sync.dma_start(
                    out_ds[0].ap()[:, h0 * TILE:(h0 + 4) * TILE],
                    stages[0][:, h0 * TILE:(h0 + 4) * TILE])
                nc.sync.dma_start(
                    out_ds[2].ap()[:, h0 * W2_:(h0 + 4) * W2_],
                    stages[2][:, h0 * W2_:(h0 + 4) * W2_])

    nc.compile()
    return nc


# ---------------------------------------------------------------- host glue
def _to_bf16(x):
    import ml_dtypes
    return np.ascontiguousarray(x, np.float32).astype(ml_dtypes.bfloat16)


def _host_tables(c_sorted, omQ):
    """U: [128, 1024] interleaved cos/sin rows, chunk-major."""
    nQ = len(omQ)
    ang = np.multiply.outer(omQ, c_sorted.astype(np.float64)) * TWO_PI  # [nQ, T]
    cosr = np.cos(ang).astype(np.float32)
    sinr = np.sin(ang).astype(np.float32)
    U = np.zeros((TILE, 2 * T), np.float32)
    for c in range(2):
        for kk in range(64):
            k = 64 * c + kk
            if k >= nQ:
                break
            U[2 * kk, c * T:(c + 1) * T] = cosr[k]
            U[2 * kk + 1, c * T:(c + 1) * T] = sinr[k]
    return U


def _pack_W(a, b, nQ):
    """packW [128, 32*128]: per (h,c) block-diag lhsT so W^T @ U = rcomb.

    rcomb[2j]   = a_j U[2j] - b_j U[2j+1]
    rcomb[2j+1] = b_j U[2j] + a_j U[2j+1]
    """
    a_pad = np.zeros((TILE, NH), np.float64)
    a_pad[:nQ] = a
    b_pad = np.zeros((TILE, NH), np.float64)
    b_pad[1:1 + b.shape[0]] = b
    W = np.zeros((TILE, 2 * NH * TILE), np.float32)
    j = np.arange(64)
    for h in range(NH):
        for c in range(2):
            base = (2 * h + c) * TILE
            k = 64 * c + j
            W[2 * j, base + 2 * j] = a_pad[k, h]
            W[2 * j + 1, base + 2 * j] = -b_pad[k, h]
            W[2 * j, base + 2 * j + 1] = b_pad[k, h]
            W[2 * j + 1, base + 2 * j + 1] = a_pad[k, h]
    return W


def kernel(centers01, mask, bias_phase, W1, b1, W2, b2):
    global _MODULE, _LAST_RESULTS
    from concourse.bass_utils import run_bass_kernel_spmd

    centers01 = np.asarray(centers01, np.float32)
    bias_phase = np.asarray(bias_phase, np.float64)
    W1 = np.asarray(W1, np.float64)
    b1 = np.asarray(b1, np.float64)
    W2 = np.asarray(W2, np.float64)
    b2 = np.asarray(b2, np.float64)

    ck = hash((bias_phase.tobytes(), W1.tobytes(), b1.tobytes(),
               W2.tobytes(), b2.tobytes()))
    if ck not in _FIT_CACHE:
        _FIT_CACHE[ck] = _fit_coefs(bias_phase, W1, b1, W2, b2)
    omQ, omP, a, b, _gridmax = _FIT_CACHE[ck]
    nQ = len(omQ)

    packW = _to_bf16(_pack_W(a, b, nQ))

    if _MODULE is None:
        _MODULE = _build_module()
    nc = _MODULE

    in_maps = []
    idxs = []
    for bi in range(N_CORES):
        c = centers01[bi]
        idx = np.argsort(c, kind="stable")
        idxs.append(idx)
        U = _host_tables(c[idx], omQ)
        in_maps.append({
            "packU": _to_bf16(U),
            "packW": packW,
        })

    res = run_bass_kernel_spmd(nc, in_maps, list(range(N_CORES)))
    _LAST_RESULTS = res

    out = np.empty((B, NH, T, T), np.float32)
    M = np.empty((NH, T, T), np.float32)
    iu = np.triu_indices(T, 1)
    for bi in range(N_CORES):
        for i in range(NT):
            Wi = (i + 1) * TILE
            raw = np.asarray(res.results[bi][f"out{i}"])
            if raw.dtype != np.uint16:
                raw = raw.view(np.uint16)
            f = (raw.astype(np.uint32) << 16).view(np.float32)
            M[:, i * TILE:(i + 1) * TILE, 0:Wi] = \
                f.reshape(TILE, NH, Wi).transpose(1, 0, 2)
        M[:, iu[0], iu[1]] = M[:, iu[1], iu[0]]
        inv = np.empty(T, np.int64)
        inv[idxs[bi]] = np.arange(T)
        out[bi] = M[:, inv][:, :, inv]
    m = np.asarray(mask, bool)
    if not m.all():
        out *= (m[:, None, :, None] & m[:, None, None, :]).astype(np.float32)
    return out


# revision 17
# speedup vs baseline: 1.0116x; 1.0116x over previous
"""Trainium2 Bass kernel for nn_RelPosRFFBias — factorized Fourier, v5 (full host mirror).

Math: per head h, bias(t,s) = g_h(|c_t - c_s|) with g_h fit as a ~125-tone
cosine+sine series.  In sorted-center order, for t >= s (lower triangle):

  g(d) = sum_k a_k cos(w_k d) + b_k sin(w_k d)
       = sum_k cos_t (a_k cos_s - b_k sin_s) + sin_t (a_k sin_s + b_k cos_s)

so ONE rank-256 matmul per (row-tile, head) with lhs = raw interleaved
cos/sin table U and rhs = rcomb = aq (.) U + bq (.) V (V = pair-swapped U)
evaluates the whole lower block-triangle, diagonal tiles included.  The host
mirrors the strict upper triangle (inter- and intra-tile) by symmetry and
undoes the sort permutation.

v5 vs v4 baseline:
 - no separate P matmuls / diag sign fixups: PE work halves (43us -> ~18us).
 - builds are 2 DVE passes per (head, chunk): tensor_scalar + fused
   scalar_tensor_tensor, both 4x-mode eligible.
 - PSUM evacuation split across Scalar (i=3, i=1) and GpSimd (i=2, i=0).
 - warmup trimmed to ~7 matmuls (p-state ramp needs ~3us, not 6.2us).
 - head-group-column-major schedule: output DMA per (row-tile, 4-head group)
   spread across SP/Act/DVE queues; short tail.
"""

import math

import numpy as np

B, T = 8, 512
RFF, NH = 16, 16
F_MIN, F_MAX = 2.0, 64.0
TWO_PI = 2.0 * math.pi

N_CORES = 8
L_PER = 1.0625
KU = 124
NQMAX = 128
TILE = 128
NT = T // TILE
FIT_LAM = 1e-5
FIT_ITERS = 14
NWARM = 5

_MODULE = None
_LAST_RESULTS = None
_FIT_CACHE = {}


# ---------------------------------------------------------------- host: fit
def _gelu64(x):
    try:
        from scipy.special import erf
    except ImportError:
        erf = np.vectorize(math.erf)
    return 0.5 * x * (1.0 + erf(x / math.sqrt(2.0)))


def _g_of_D(D, phase, W1, b1, W2, b2, freqs):
    arg = TWO_PI * D[:, None] * freqs[None, :] + phase[None, :]
    feats = np.concatenate([np.sin(arg), np.cos(arg)], axis=-1)
    return _gelu64(feats @ W1 + b1) @ W2 + b2


def _tone_grid():
    freqs = np.logspace(math.log10(F_MIN), math.log10(F_MAX), RFF).astype(np.float64)
    uni = np.arange(KU) / L_PER
    cut = uni[-1]
    cand = sorted(set(
        round(f, 6)
        for f in np.concatenate([(freqs[:, None] + freqs[None, :]).ravel(), 2 * freqs])
        if cut + 0.2 < f < 145.0
    ))
    omQ = np.concatenate([uni, np.asarray(cand[: NQMAX - KU], dtype=np.float64)])
    omQ.sort()
    omP = omQ[1:min(len(omQ), 129)]
    return freqs, omQ, omP


def _fit_coefs(phase, W1, b1, W2, b2):
    freqs, omQ, omP = _tone_grid()
    NG = 32768
    Dg = (np.arange(NG) + 0.5) / NG
    G = _g_of_D(Dg, phase, W1, b1, W2, b2, freqs)
    Phi = np.concatenate(
        [np.cos(Dg[:, None] * TWO_PI * omQ[None, :]),
         np.sin(Dg[:, None] * TWO_PI * omP[None, :])], axis=1)
    lam = FIT_LAM * NG
    w = np.ones(NG)
    best = None
    for _ in range(FIT_ITERS):
        Pw = Phi * w[:, None]
        A = Pw.T @ Phi
        A[np.diag_indices_from(A)] += lam
        coef = np.linalg.solve(A, Pw.T @ G)
        res = np.abs(Phi @ coef - G).max(axis=1)
        mx = res.max()
        if best is None or mx < best[0]:
            best = (mx, coef.copy())
        w = w * (0.05 + res / mx)
        w = np.maximum(w / w.mean(), 1e-6)
    mx, coef = best
    return omQ, omP, coef[: len(omQ)], coef[len(omQ):], mx


# ---------------------------------------------------------------- device
def _build_module():
    import concourse.tile as tile
    from concourse import bacc, mybir
    from contextlib import ExitStack

    f32 = mybir.dt.float32
    bf16 = mybir.dt.bfloat16
    Alu = mybir.AluOpType
    Act = mybir.ActivationFunctionType

    nc = bacc.Bacc("TRN2", target_bir_lowering=False, debug=False)

    # packU: raw interleaved cos/sin table, chunk-major [U0 | U1]
    packU_d = nc.dram_tensor("packU", [TILE, 2 * T], bf16, kind="ExternalInput")
    # packW: per (h, c) 2x2-block-diagonal rotation/scale lhsT [128,128],
    # col block at (2h+c)*128.  rcomb_{h,c} = packW_{h,c}^T @ U_c on the PE.
    packW_d = nc.dram_tensor("packW", [TILE, 2 * NH * TILE], bf16,
                             kind="ExternalInput")
    # compact block-lower-triangle outputs, head-major per row-tile i
    out_ds = [nc.dram_tensor(f"out{i}", [TILE, NH * (i + 1) * TILE], bf16,
                             kind="ExternalOutput") for i in range(NT)]

    with tile.TileContext(nc) as tc:
        with ExitStack() as ctx:
            const = ctx.enter_context(tc.tile_pool(name="const", bufs=1))
            rhspool = ctx.enter_context(tc.tile_pool(name="rhs", bufs=1))
            stpool = ctx.enter_context(tc.tile_pool(name="stage", bufs=1))
            # PSUM: pb = [128,1024] (2 banks) x2 bufs for i3/i2 head pairs;
            # pv = [128,1024] (2 banks) x1 buf for the two i1 pairs;
            # pr = [128,512] x2 bufs for rcomb builds + i0 + warmup.
            # 4 + 2 + 2 = 8 banks.
            pb = ctx.enter_context(tc.tile_pool(name="pb", bufs=2, space="PSUM"))
            pv = ctx.enter_context(tc.tile_pool(name="pv", bufs=1, space="PSUM"))
            pr = ctx.enter_context(tc.tile_pool(name="pr", bufs=2, space="PSUM"))

            packU = const.tile([TILE, 2 * T], bf16, tag="packU")
            nc.sync.dma_start(packU[:], packU_d.ap())
            packW = const.tile([TILE, 2 * NH * TILE], bf16, tag="packW")
            nc.sync.dma_start(packW[:, 0:NH * TILE], packW_d.ap()[:, 0:NH * TILE])
            nc.sync.dma_start(packW[:, NH * TILE:], packW_d.ap()[:, NH * TILE:])

            # PE warm-up: ~3us of dummy matmuls to finish the p-state ramp
            # while the input DMAs land.
            warm_sb = const.tile([TILE, T], bf16, tag="warm")
            nc.vector.memset(warm_sb[:], 0)
            for _ in range(NWARM):
                warm_ps = pr.tile([TILE, T], f32, tag="pr")
                nc.tensor.matmul(warm_ps[:], warm_sb[:, 0:TILE], warm_sb[:],
                                 start=True, stop=True)

            # rcomb: [128, NH*1024], per head h chunk c at h*1024 + c*512,
            # built on the PE: rcomb_{h,c} = packW_{h,c}^T @ U_c, then
            # evacuated PSUM->SBUF by DVE (mostly) / Scalar.
            rcomb = rhspool.tile([TILE, NH * 2 * T], bf16, tag="rcomb",
                                 name="rcomb")

            def build(h, last_g=False):
                for c in range(2):
                    col = 2 * h + c
                    ps = pr.tile([TILE, T], f32, tag="pr")
                    nc.tensor.matmul(
                        ps[:], packW[:, col * TILE:(col + 1) * TILE],
                        packU[:, c * T:(c + 1) * T], start=True, stop=True)
                    dst = rcomb[:, h * 2 * T + c * T: h * 2 * T + (c + 1) * T]
                    if col % 10 == 1:
                        nc.scalar.activation(dst, ps[:], Act.Identity)
                    else:
                        nc.vector.tensor_copy(dst, ps[:])

            for h in range(6):
                build(h)

            stages = [stpool.tile([TILE, NH * (i + 1) * TILE], bf16,
                                  tag=f"stage{i}", name=f"stage{i}")
                      for i in range(NT)]
            rc_v = rcomb[:].rearrange("p (h x) -> p h x", h=NH)

            def rslice(h, c, n):
                return rcomb[:, h * 2 * T + c * T: h * 2 * T + c * T + n]

            def evac(dst, src, g, k):
                # last head group: split evacuation across both engines to
                # shorten the tail (DVE is otherwise idle by then)
                if g == 3 and k % 2 == 0:
                    nc.vector.tensor_copy(dst, src)
                else:
                    nc.scalar.activation(dst, src, Act.Identity)

            W2_ = 3 * TILE
            W1_ = 2 * TILE

            def sec_i3(g, h0, nxt):
                # i = 3: head pairs, 2 banks per psum tile, N=512 each
                for m, hp in enumerate((h0, h0 + 2)):
                    ps = pb.tile([TILE, 2 * T], f32, tag="pb")
                    for j in range(2):
                        for c in range(2):
                            nc.tensor.matmul(
                                ps[:, j * T:(j + 1) * T],
                                packU[:, c * T + 3 * TILE: c * T + 4 * TILE],
                                rslice(hp + j, c, T),
                                start=(c == 0), stop=(c == 1),
                                skip_group_check=True)
                    if m == 0 and nxt:
                        build(nxt[0])
                    evac(stages[3][:, hp * T:(hp + 2) * T], ps[:], g, m)

            def sec_i10(g, h0, nxt):
                # i = 1 (two heads per bank) + i = 0 (four heads in one bank)
                psv = pv.tile([TILE, 2 * T], f32, tag="pv")
                for k, hp in enumerate((h0, h0 + 2)):
                    for c in range(2):
                        nc.tensor.matmul(
                            psv[:, k * T:(k + 1) * T],
                            packU[:, c * T + TILE: c * T + 2 * TILE],
                            rc_v[:, hp:hp + 2, c * T: c * T + W1_],
                            start=(c == 0), stop=(c == 1),
                            skip_group_check=True)
                ps0 = pr.tile([TILE, T], f32, tag="pr")
                for c in range(2):
                    nc.tensor.matmul(
                        ps0[:],
                        packU[:, c * T: c * T + TILE],
                        rc_v[:, h0:h0 + 4, c * T: c * T + TILE],
                        start=(c == 0), stop=(c == 1))
                if len(nxt) > 2:
                    build(nxt[2])
                evac(stages[1][:, h0 * W1_:(h0 + 4) * W1_], psv[:], g, 1)
                evac(stages[0][:, h0 * TILE:(h0 + 4) * TILE], ps0[:], g, 0)

            def sec_i2(g, h0, nxt):
                # i = 2: head pairs, N=384 into each bank of a 2-bank tile
                for m, hp in enumerate((h0, h0 + 2)):
                    ps = pb.tile([TILE, 2 * T], f32, tag="pb")
                    for j in range(2):
                        for c in range(2):
                            nc.tensor.matmul(
                                ps[:, j * T:j * T + W2_],
                                packU[:, c * T + 2 * TILE: c * T + 3 * TILE],
                                rslice(hp + j, c, W2_),
                                start=(c == 0), stop=(c == 1),
                                skip_group_check=True)
                    if m == 0 and len(nxt) > 3:
                        build(nxt[3])
                    evac(stages[2][:, hp * W2_:(hp + 2) * W2_]
                         .rearrange("p (b s) -> p b s", b=2),
                         ps[:].rearrange("p (b s) -> p b s", b=2)[:, :, 0:W2_],
                         g, m)

            def dma_out(i, h0, n=4):
                Wi = (i + 1) * TILE
                nc.sync.dma_start(
                    out_ds[i].ap()[:, h0 * Wi:(h0 + n) * Wi],
                    stages[i][:, h0 * Wi:(h0 + n) * Wi])

            for g in range(4):
                h0 = 4 * g
                nxt = list(range(h0 + 6, min(h0 + 10, NH))) if g < 3 else []
                sec_i3(g, h0, nxt)
                if len(nxt) > 1:
                    build(nxt[1])
                if g < 3:
                    sec_i10(g, h0, nxt)
                    dma_out(3, h0)
                    sec_i2(g, h0, nxt)
                    dma_out(1, h0)
                    dma_out(0, h0)
                    dma_out(2, h0)
                else:
                    # last group: big tiles first so the DMA tail is short
                    dma_out(3, h0)
                    sec_i2(g, h0, nxt)
                    dma_out(2, h0)
                    sec_i10(g, h0, nxt)
                    dma_out(1, h0)
                    dma_out(0, h0)

    nc.compile()
    return nc


# ---------------------------------------------------------------- host glue
def _to_bf16(x):
    import ml_dtypes
    return np.ascontiguousarray(x, np.float32).astype(ml_dtypes.bfloat16)


def _host_tables(c_sorted, omQ):
    """U: [128, 1024] interleaved cos/sin rows, chunk-major."""
    nQ = len(omQ)
    ang = np.multiply.outer(omQ, c_sorted.astype(np.float64)) * TWO_PI  # [nQ, T]
    cosr = np.cos(ang).astype(np.float32)
    sinr = np.sin(ang).astype(np.float32)
    U = np.zeros((TILE, 2 * T), np.float32)
    for c in range(2):
        for kk in range(64):
            k = 64 * c + kk
            if k >= nQ:
                break
            U[2 * kk, c * T:(c + 1) * T] = cosr[k]
            U[2 * kk + 1, c * T:(c + 1) * T] = sinr[k]
    return U


def _pack_W(a, b, nQ):
    """packW [128, 32*128]: per (h,c) block-diag lhsT so W^T @ U = rcomb.

    rcomb[2j]   = a_j U[2j] - b_j U[2j+1]
    rcomb[2j+1] = b_j U[2j] + a_j U[2j+1]
    """
    a_pad = np.zeros((TILE, NH), np.float64)
    a_pad[:nQ] = a
    b_pad = np.zeros((TILE, NH), np.float64)
    b_pad[1:1 + b.shape[0]] = b
    W = np.zeros((TILE, 2 * NH * TILE), np.float32)
    j = np.arange(64)
    for h in range(NH):
        for c in range(2):
            base = (2 * h + c) * TILE
            k = 64 * c + j
            W[2 * j, base + 2 * j] = a_pad[k, h]
            W[2 * j + 1, base + 2 * j] = -b_pad[k, h]
            W[2 * j, base + 2 * j + 1] = b_pad[k, h]
            W[2 * j + 1, base + 2 * j + 1] = a_pad[k, h]
    return W


def kernel(centers01, mask, bias_phase, W1, b1, W2, b2):
    global _MODULE, _LAST_RESULTS
    from concourse.bass_utils import run_bass_kernel_spmd

    centers01 = np.asarray(centers01, np.float32)
    bias_phase = np.asarray(bias_phase, np.float64)
    W1 = np.asarray(W1, np.float64)
    b1 = np.asarray(b1, np.float64)
    W2 = np.asarray(W2, np.float64)
    b2 = np.asarray(b2, np.float64)

    ck = hash((bias_phase.tobytes(), W1.tobytes(), b1.tobytes(),
               W2.tobytes(), b2.tobytes()))
    if ck not in _FIT_CACHE:
        _FIT_CACHE[ck] = _fit_coefs(bias_phase, W1, b1, W2, b2)
    omQ, omP, a, b, _gridmax = _FIT_CACHE[ck]
    nQ = len(omQ)

    packW = _to_bf16(_pack_W(a, b, nQ))

    if _MODULE is None:
        _MODULE = _build_module()
    nc = _MODULE

    in_maps = []
    idxs = []
    for bi in range(N_CORES):
        c = centers01[bi]
        idx = np.argsort(c, kind="stable")
        idxs.append(idx)
        U = _host_tables(c[idx], omQ)
        in_maps.append({
            "packU": _to_bf16(U),
            "packW": packW,
        })

    res = run_bass_kernel_spmd(nc, in_maps, list(range(N_CORES)))
    _LAST_RESULTS = res

    out = np.empty((B, NH, T, T), np.float32)
    M = np.empty((NH, T, T), np.float32)
    iu = np.triu_indices(T, 1)
    for bi in range(N_CORES):
        for i in range(NT):
            Wi = (i + 1) * TILE
            raw = np.asarray(res.results[bi][f"out{i}"])
            if raw.dtype != np.uint16:
                raw = raw.view(np.uint16)
            f = (raw.astype(np.uint32) << 16).view(np.float32)
            M[:, i * TILE:(i + 1) * TILE, 0:Wi] = \
                f.reshape(TILE, NH, Wi).transpose(1, 0, 2)
        M[:, iu[0], iu[1]] = M[:, iu[1], iu[0]]
        inv = np.empty(T, np.int64)
        inv[idxs[bi]] = np.arange(T)
        out[bi] = M[:, inv][:, :, inv]
    m = np.asarray(mask, bool)
    if not m.all():
        out *= (m[:, None, :, None] & m[:, None, None, :]).astype(np.float32)
    return out


# revision 19
# speedup vs baseline: 1.0418x; 1.0299x over previous
"""Trainium2 Bass kernel for nn_RelPosRFFBias — factorized Fourier, v5 (full host mirror).

Math: per head h, bias(t,s) = g_h(|c_t - c_s|) with g_h fit as a ~125-tone
cosine+sine series.  In sorted-center order, for t >= s (lower triangle):

  g(d) = sum_k a_k cos(w_k d) + b_k sin(w_k d)
       = sum_k cos_t (a_k cos_s - b_k sin_s) + sin_t (a_k sin_s + b_k cos_s)

so ONE rank-256 matmul per (row-tile, head) with lhs = raw interleaved
cos/sin table U and rhs = rcomb = aq (.) U + bq (.) V (V = pair-swapped U)
evaluates the whole lower block-triangle, diagonal tiles included.  The host
mirrors the strict upper triangle (inter- and intra-tile) by symmetry and
undoes the sort permutation.

v5 vs v4 baseline:
 - no separate P matmuls / diag sign fixups: PE work halves (43us -> ~18us).
 - builds are 2 DVE passes per (head, chunk): tensor_scalar + fused
   scalar_tensor_tensor, both 4x-mode eligible.
 - PSUM evacuation split across Scalar (i=3, i=1) and GpSimd (i=2, i=0).
 - warmup trimmed to ~7 matmuls (p-state ramp needs ~3us, not 6.2us).
 - head-group-column-major schedule: output DMA per (row-tile, 4-head group)
   spread across SP/Act/DVE queues; short tail.
"""

import math

import numpy as np

B, T = 8, 512
RFF, NH = 16, 16
F_MIN, F_MAX = 2.0, 64.0
TWO_PI = 2.0 * math.pi

N_CORES = 8
L_PER = 1.0625
KU = 124
NQMAX = 128
TILE = 128
NT = T // TILE
FIT_LAM = 1e-5
FIT_ITERS = 14
NWARM = 5

_MODULE = None
_LAST_RESULTS = None
_FIT_CACHE = {}


# ---------------------------------------------------------------- host: fit
def _gelu64(x):
    try:
        from scipy.special import erf
    except ImportError:
        erf = np.vectorize(math.erf)
    return 0.5 * x * (1.0 + erf(x / math.sqrt(2.0)))


def _g_of_D(D, phase, W1, b1, W2, b2, freqs):
    arg = TWO_PI * D[:, None] * freqs[None, :] + phase[None, :]
    feats = np.concatenate([np.sin(arg), np.cos(arg)], axis=-1)
    return _gelu64(feats @ W1 + b1) @ W2 + b2


def _tone_grid():
    freqs = np.logspace(math.log10(F_MIN), math.log10(F_MAX), RFF).astype(np.float64)
    uni = np.arange(KU) / L_PER
    cut = uni[-1]
    cand = sorted(set(
        round(f, 6)
        for f in np.concatenate([(freqs[:, None] + freqs[None, :]).ravel(), 2 * freqs])
        if cut + 0.2 < f < 145.0
    ))
    omQ = np.concatenate([uni, np.asarray(cand[: NQMAX - KU], dtype=np.float64)])
    omQ.sort()
    omP = omQ[1:min(len(omQ), 129)]
    return freqs, omQ, omP


def _fit_coefs(phase, W1, b1, W2, b2):
    freqs, omQ, omP = _tone_grid()
    NG = 32768
    Dg = (np.arange(NG) + 0.5) / NG
    G = _g_of_D(Dg, phase, W1, b1, W2, b2, freqs)
    Phi = np.concatenate(
        [np.cos(Dg[:, None] * TWO_PI * omQ[None, :]),
         np.sin(Dg[:, None] * TWO_PI * omP[None, :])], axis=1)
    lam = FIT_LAM * NG
    w = np.ones(NG)
    best = None
    for _ in range(FIT_ITERS):
        Pw = Phi * w[:, None]
        A = Pw.T @ Phi
        A[np.diag_indices_from(A)] += lam
        coef = np.linalg.solve(A, Pw.T @ G)
        res = np.abs(Phi @ coef - G).max(axis=1)
        mx = res.max()
        if best is None or mx < best[0]:
            best = (mx, coef.copy())
        w = w * (0.05 + res / mx)
        w = np.maximum(w / w.mean(), 1e-6)
    mx, coef = best
    return omQ, omP, coef[: len(omQ)], coef[len(omQ):], mx


# ---------------------------------------------------------------- device
def _build_module():
    import concourse.tile as tile
    from concourse import bacc, mybir
    from contextlib import ExitStack

    f32 = mybir.dt.float32
    bf16 = mybir.dt.bfloat16
    Alu = mybir.AluOpType
    Act = mybir.ActivationFunctionType

    nc = bacc.Bacc("TRN2", target_bir_lowering=False, debug=False)

    # packU: raw interleaved cos/sin table, chunk-major [U0 | U1]
    packU_d = nc.dram_tensor("packU", [TILE, 2 * T], bf16, kind="ExternalInput")
    # packW: per (h, c) 2x2-block-diagonal rotation/scale lhsT [128,128],
    # col block at (2h+c)*128.  rcomb_{h,c} = packW_{h,c}^T @ U_c on the PE.
    packW_d = nc.dram_tensor("packW", [TILE, 2 * NH * TILE], bf16,
                             kind="ExternalInput")
    # compact block-lower-triangle outputs, head-major per row-tile i
    out_ds = [nc.dram_tensor(f"out{i}", [TILE, NH * (i + 1) * TILE], bf16,
                             kind="ExternalOutput") for i in range(NT)]

    with tile.TileContext(nc) as tc:
        with ExitStack() as ctx:
            const = ctx.enter_context(tc.tile_pool(name="const", bufs=1))
            rhspool = ctx.enter_context(tc.tile_pool(name="rhs", bufs=1))
            stpool = ctx.enter_context(tc.tile_pool(name="stage", bufs=1))
            # PSUM: pb = [128,1024] (2 banks) x3 bufs for i3/i2 head pairs,
            # the i1 pair-of-pairs, and warmups; pr = [128,512] x2 bufs for
            # rcomb builds + i0.  6 + 2 = 8 banks.
            pb = ctx.enter_context(tc.tile_pool(name="pb", bufs=3, space="PSUM"))
            pr = ctx.enter_context(tc.tile_pool(name="pr", bufs=2, space="PSUM"))

            packW = const.tile([TILE, 2 * NH * TILE], bf16, tag="packW")
            packU = const.tile([TILE, 2 * T], bf16, tag="packU")
            nc.sync.dma_start(packW[:, 0:NH * TILE], packW_d.ap()[:, 0:NH * TILE])
            nc.sync.dma_start(packU[:], packU_d.ap())
            nc.sync.dma_start(packW[:, NH * TILE:], packW_d.ap()[:, NH * TILE:])

            # PE warm-up on the pb ring (no readers -> never blocks): keeps
            # the PE p-state ramp alive while input DMAs land and between
            # the early build matmuls, whose PSUM ring waits on evacuation.
            warm_sb = const.tile([TILE, T], bf16, tag="warm")
            nc.vector.memset(warm_sb[:], 0)

            def warm(n=1):
                for _ in range(n):
                    warm_ps = pb.tile([TILE, 2 * T], f32, tag="pb")
                    nc.tensor.matmul(warm_ps[:, 0:T], warm_sb[:, 0:TILE],
                                     warm_sb[:], start=True, stop=True)

            # rcomb: [128, NH*1024], per head h chunk c at h*1024 + c*512,
            # built on the PE: rcomb_{h,c} = packW_{h,c}^T @ U_c, then
            # evacuated PSUM->SBUF by DVE / Scalar.
            rcomb = rhspool.tile([TILE, NH * 2 * T], bf16, tag="rcomb",
                                 name="rcomb")

            def build(h, pre=False):
                for c in range(2):
                    col = 2 * h + c
                    ps = pr.tile([TILE, T], f32, tag="pr")
                    nc.tensor.matmul(
                        ps[:], packW[:, col * TILE:(col + 1) * TILE],
                        packU[:, c * T:(c + 1) * T], start=True, stop=True)
                    if pre:
                        warm()
                    dst = rcomb[:, h * 2 * T + c * T: h * 2 * T + (c + 1) * T]
                    if (col % 2 == 1 if pre else col % 10 == 1):
                        nc.scalar.activation(dst, ps[:], Act.Identity)
                    else:
                        nc.vector.tensor_copy(dst, ps[:])

            warm(NWARM)
            for h in range(6):
                build(h, pre=True)

            stages = [stpool.tile([TILE, NH * (i + 1) * TILE], bf16,
                                  tag=f"stage{i}", name=f"stage{i}")
                      for i in range(NT)]
            rc_v = rcomb[:].rearrange("p (h x) -> p h x", h=NH)

            def rslice(h, c, n):
                return rcomb[:, h * 2 * T + c * T: h * 2 * T + c * T + n]

            def evac(dst, src, g, k):
                # last head group: split evacuation across both engines to
                # shorten the tail (DVE is otherwise idle by then)
                if g == 3 and k % 2 == 0:
                    nc.vector.tensor_copy(dst, src)
                else:
                    nc.scalar.activation(dst, src, Act.Identity)

            W2_ = 3 * TILE
            W1_ = 2 * TILE

            def sec_i3(g, h0, nxt):
                # i = 3: head pairs, 2 banks per psum tile, N=512 each
                for m, hp in enumerate((h0, h0 + 2)):
                    ps = pb.tile([TILE, 2 * T], f32, tag="pb")
                    for j in range(2):
                        for c in range(2):
                            nc.tensor.matmul(
                                ps[:, j * T:(j + 1) * T],
                                packU[:, c * T + 3 * TILE: c * T + 4 * TILE],
                                rslice(hp + j, c, T),
                                start=(c == 0), stop=(c == 1),
                                skip_group_check=True)
                    if m == 0 and nxt:
                        build(nxt[0])
                    evac(stages[3][:, hp * T:(hp + 2) * T], ps[:], g, m)

            def sec_i10(g, h0, nxt):
                # i = 1 (two heads per bank) + i = 0 (four heads in one bank)
                psv = pb.tile([TILE, 2 * T], f32, tag="pb")
                for k, hp in enumerate((h0, h0 + 2)):
                    for c in range(2):
                        nc.tensor.matmul(
                            psv[:, k * T:(k + 1) * T],
                            packU[:, c * T + TILE: c * T + 2 * TILE],
                            rc_v[:, hp:hp + 2, c * T: c * T + W1_],
                            start=(c == 0), stop=(c == 1),
                            skip_group_check=True)
                ps0 = pr.tile([TILE, T], f32, tag="pr")
                for c in range(2):
                    nc.tensor.matmul(
                        ps0[:],
                        packU[:, c * T: c * T + TILE],
                        rc_v[:, h0:h0 + 4, c * T: c * T + TILE],
                        start=(c == 0), stop=(c == 1))
                if len(nxt) > 2:
                    build(nxt[2])
                evac(stages[1][:, h0 * W1_:(h0 + 4) * W1_], psv[:], g, 1)
                evac(stages[0][:, h0 * TILE:(h0 + 4) * TILE], ps0[:], g, 0)

            def sec_i2(g, h0, nxt):
                # i = 2: head pairs, N=384 into each bank of a 2-bank tile
                for m, hp in enumerate((h0, h0 + 2)):
                    ps = pb.tile([TILE, 2 * T], f32, tag="pb")
                    for j in range(2):
                        for c in range(2):
                            nc.tensor.matmul(
                                ps[:, j * T:j * T + W2_],
                                packU[:, c * T + 2 * TILE: c * T + 3 * TILE],
                                rslice(hp + j, c, W2_),
                                start=(c == 0), stop=(c == 1),
                                skip_group_check=True)
                    if m == 0 and len(nxt) > 3:
                        build(nxt[3])
                    evac(stages[2][:, hp * W2_:(hp + 2) * W2_]
                         .rearrange("p (b s) -> p b s", b=2),
                         ps[:].rearrange("p (b s) -> p b s", b=2)[:, :, 0:W2_],
                         g, m)

            def dma_out(i, h0, n=4):
                Wi = (i + 1) * TILE
                nc.sync.dma_start(
                    out_ds[i].ap()[:, h0 * Wi:(h0 + n) * Wi],
                    stages[i][:, h0 * Wi:(h0 + n) * Wi])

            for g in range(4):
                h0 = 4 * g
                nxt = list(range(h0 + 6, min(h0 + 10, NH))) if g < 3 else []
                sec_i3(g, h0, nxt)
                if len(nxt) > 1:
                    build(nxt[1])
                if g < 3:
                    sec_i10(g, h0, nxt)
                    dma_out(3, h0)
                    sec_i2(g, h0, nxt)
                    dma_out(1, h0)
                    dma_out(0, h0)
                    dma_out(2, h0)
                else:
                    # last group: big tiles first so the DMA tail is short
                    dma_out(3, h0)
                    sec_i2(g, h0, nxt)
                    dma_out(2, h0)
                    sec_i10(g, h0, nxt)
                    dma_out(1, h0)
                    dma_out(0, h0)

    nc.compile()
    return nc


# ---------------------------------------------------------------- host glue
def _to_bf16(x):
    import ml_dtypes
    return np.ascontiguousarray(x, np.float32).astype(ml_dtypes.bfloat16)


def _host_tables(c_sorted, omQ):
    """U: [128, 1024] interleaved cos/sin rows, chunk-major."""
    nQ = len(omQ)
    ang = np.multiply.outer(omQ, c_sorted.astype(np.float64)) * TWO_PI  # [nQ, T]
    cosr = np.cos(ang).astype(np.float32)
    sinr = np.sin(ang).astype(np.float32)
    U = np.zeros((TILE, 2 * T), np.float32)
    for c in range(2):
        for kk in range(64):
            k = 64 * c + kk
            if k >= nQ:
                break
            U[2 * kk, c * T:(c + 1) * T] = cosr[k]
            U[2 * kk + 1, c * T:(c + 1) * T] = sinr[k]
    return U


def _pack_W(a, b, nQ):
    """packW [128, 32*128]: per (h,c) block-diag lhsT so W^T @ U = rcomb.

    rcomb[2j]   = a_j U[2j] - b_j U[2j+1]
    rcomb[2j+1] = b_j U[2j] + a_j U[2j+1]
    """
    a_pad = np.zeros((TILE, NH), np.float64)
    a_pad[:nQ] = a
    b_pad = np.zeros((TILE, NH), np.float64)
    b_pad[1:1 + b.shape[0]] = b
    W = np.zeros((TILE, 2 * NH * TILE), np.float32)
    j = np.arange(64)
    for h in range(NH):
        for c in range(2):
            base = (2 * h + c) * TILE
            k = 64 * c + j
            W[2 * j, base + 2 * j] = a_pad[k, h]
            W[2 * j + 1, base + 2 * j] = -b_pad[k, h]
            W[2 * j, base + 2 * j + 1] = b_pad[k, h]
            W[2 * j + 1, base + 2 * j + 1] = a_pad[k, h]
    return W


def kernel(centers01, mask, bias_phase, W1, b1, W2, b2):
    global _MODULE, _LAST_RESULTS
    from concourse.bass_utils import run_bass_kernel_spmd

    centers01 = np.asarray(centers01, np.float32)
    bias_phase = np.asarray(bias_phase, np.float64)
    W1 = np.asarray(W1, np.float64)
    b1 = np.asarray(b1, np.float64)
    W2 = np.asarray(W2, np.float64)
    b2 = np.asarray(b2, np.float64)

    ck = hash((bias_phase.tobytes(), W1.tobytes(), b1.tobytes(),
               W2.tobytes(), b2.tobytes()))
    if ck not in _FIT_CACHE:
        _FIT_CACHE[ck] = _fit_coefs(bias_phase, W1, b1, W2, b2)
    omQ, omP, a, b, _gridmax = _FIT_CACHE[ck]
    nQ = len(omQ)

    packW = _to_bf16(_pack_W(a, b, nQ))

    if _MODULE is None:
        _MODULE = _build_module()
    nc = _MODULE

    in_maps = []
    idxs = []
    for bi in range(N_CORES):
        c = centers01[bi]
        idx = np.argsort(c, kind="stable")
        idxs.append(idx)
        U = _host_tables(c[idx], omQ)
        in_maps.append({
            "packU": _to_bf16(U),
            "packW": packW,
        })

    res = run_bass_kernel_spmd(nc, in_maps, list(range(N_CORES)))
    _LAST_RESULTS = res

    out = np.empty((B, NH, T, T), np.float32)
    M = np.empty((NH, T, T), np.float32)
    iu = np.triu_indices(T, 1)
    for bi in range(N_CORES):
        for i in range(NT):
            Wi = (i + 1) * TILE
            raw = np.asarray(res.results[bi][f"out{i}"])
            if raw.dtype != np.uint16:
                raw = raw.view(np.uint16)
            f = (raw.astype(np.uint32) << 16).view(np.float32)
            M[:, i * TILE:(i + 1) * TILE, 0:Wi] = \
                f.reshape(TILE, NH, Wi).transpose(1, 0, 2)
        M[:, iu[0], iu[1]] = M[:, iu[1], iu[0]]
        inv = np.empty(T, np.int64)
        inv[idxs[bi]] = np.arange(T)
        out[bi] = M[:, inv][:, :, inv]
    m = np.asarray(mask, bool)
    if not m.all():
        out *= (m[:, None, :, None] & m[:, None, None, :]).astype(np.float32)
    return out


# revision 30
# speedup vs baseline: 1.1000x; 1.0558x over previous
"""Trainium2 Bass kernel for nn_RelPosRFFBias — factorized Fourier, v5 (full host mirror).

Math: per head h, bias(t,s) = g_h(|c_t - c_s|) with g_h fit as a ~125-tone
cosine+sine series.  In sorted-center order, for t >= s (lower triangle):

  g(d) = sum_k a_k cos(w_k d) + b_k sin(w_k d)
       = sum_k cos_t (a_k cos_s - b_k sin_s) + sin_t (a_k sin_s + b_k cos_s)

so ONE rank-256 matmul per (row-tile, head) with lhs = raw interleaved
cos/sin table U and rhs = rcomb = aq (.) U + bq (.) V (V = pair-swapped U)
evaluates the whole lower block-triangle, diagonal tiles included.  The host
mirrors the strict upper triangle (inter- and intra-tile) by symmetry and
undoes the sort permutation.

v5 vs v4 baseline:
 - no separate P matmuls / diag sign fixups: PE work halves (43us -> ~18us).
 - builds are 2 DVE passes per (head, chunk): tensor_scalar + fused
   scalar_tensor_tensor, both 4x-mode eligible.
 - PSUM evacuation split across Scalar (i=3, i=1) and GpSimd (i=2, i=0).
 - warmup trimmed to ~7 matmuls (p-state ramp needs ~3us, not 6.2us).
 - head-group-column-major schedule: output DMA per (row-tile, 4-head group)
   spread across SP/Act/DVE queues; short tail.
"""

import math

import numpy as np

B, T = 8, 512
RFF, NH = 16, 16
F_MIN, F_MAX = 2.0, 64.0
TWO_PI = 2.0 * math.pi

N_CORES = 8
L_PER = 1.0625
KU = 124
NQMAX = 128
TILE = 128
NT = T // TILE
FIT_LAM = 1e-5
FIT_ITERS = 14
NWARM = 5

_MODULE = None
_LAST_RESULTS = None
_FIT_CACHE = {}


# ---------------------------------------------------------------- host: fit
def _gelu64(x):
    try:
        from scipy.special import erf
    except ImportError:
        erf = np.vectorize(math.erf)
    return 0.5 * x * (1.0 + erf(x / math.sqrt(2.0)))


def _g_of_D(D, phase, W1, b1, W2, b2, freqs):
    arg = TWO_PI * D[:, None] * freqs[None, :] + phase[None, :]
    feats = np.concatenate([np.sin(arg), np.cos(arg)], axis=-1)
    return _gelu64(feats @ W1 + b1) @ W2 + b2


def _tone_grid():
    freqs = np.logspace(math.log10(F_MIN), math.log10(F_MAX), RFF).astype(np.float64)
    uni = np.arange(KU) / L_PER
    cut = uni[-1]
    cand = sorted(set(
        round(f, 6)
        for f in np.concatenate([(freqs[:, None] + freqs[None, :]).ravel(), 2 * freqs])
        if cut + 0.2 < f < 145.0
    ))
    omQ = np.concatenate([uni, np.asarray(cand[: NQMAX - KU], dtype=np.float64)])
    omQ.sort()
    omP = omQ[1:min(len(omQ), 129)]
    return freqs, omQ, omP


def _fit_coefs(phase, W1, b1, W2, b2):
    freqs, omQ, omP = _tone_grid()
    NG = 32768
    Dg = (np.arange(NG) + 0.5) / NG
    G = _g_of_D(Dg, phase, W1, b1, W2, b2, freqs)
    Phi = np.concatenate(
        [np.cos(Dg[:, None] * TWO_PI * omQ[None, :]),
         np.sin(Dg[:, None] * TWO_PI * omP[None, :])], axis=1)
    lam = FIT_LAM * NG
    w = np.ones(NG)
    best = None
    for _ in range(FIT_ITERS):
        Pw = Phi * w[:, None]
        A = Pw.T @ Phi
        A[np.diag_indices_from(A)] += lam
        coef = np.linalg.solve(A, Pw.T @ G)
        res = np.abs(Phi @ coef - G).max(axis=1)
        mx = res.max()
        if best is None or mx < best[0]:
            best = (mx, coef.copy())
        w = w * (0.05 + res / mx)
        w = np.maximum(w / w.mean(), 1e-6)
    mx, coef = best
    return omQ, omP, coef[: len(omQ)], coef[len(omQ):], mx


# ---------------------------------------------------------------- device
def _build_module():
    import concourse.tile as tile
    from concourse import bacc, mybir
    from contextlib import ExitStack

    f32 = mybir.dt.float32
    bf16 = mybir.dt.bfloat16
    Alu = mybir.AluOpType
    Act = mybir.ActivationFunctionType

    nc = bacc.Bacc("TRN2", target_bir_lowering=False, debug=False)

    # packU: raw interleaved cos/sin table, chunk-major [U0 | U1]
    packU_d = nc.dram_tensor("packU", [TILE, 2 * T], bf16, kind="ExternalInput")
    # packW: per (h, c) 2x2-block-diagonal rotation/scale lhsT [128,128],
    # col block at (2h+c)*128.  rcomb_{h,c} = packW_{h,c}^T @ U_c on the PE.
    packW_d = nc.dram_tensor("packW", [TILE, 2 * NH * TILE], bf16,
                             kind="ExternalInput")
    # compact block-lower-triangle outputs, head-major per row-tile i
    out_ds = [nc.dram_tensor(f"out{i}", [TILE, NH * (i + 1) * TILE], bf16,
                             kind="ExternalOutput") for i in range(NT)]

    with tile.TileContext(nc) as tc:
        with ExitStack() as ctx:
            const = ctx.enter_context(tc.tile_pool(name="const", bufs=1))
            rhspool = ctx.enter_context(tc.tile_pool(name="rhs", bufs=1))
            stpool = ctx.enter_context(tc.tile_pool(name="stage", bufs=1))
            # PSUM: pb = [128,1024] (2 banks) x3 bufs for i3/i2 head pairs,
            # the i1 pair-of-pairs, and warmups; pr = [128,512] x2 bufs for
            # rcomb builds + i0.  6 + 2 = 8 banks.
            pb = ctx.enter_context(tc.tile_pool(name="pb", bufs=3, space="PSUM"))
            pr = ctx.enter_context(tc.tile_pool(name="pr", bufs=2, space="PSUM"))

            packW = const.tile([TILE, 2 * NH * TILE], bf16, tag="packW")
            packU = const.tile([TILE, 2 * T], bf16, tag="packU")
            # first bytes on the critical path: W blocks for h0/h1, then U,
            # then the rest of W
            nc.sync.dma_start(packW[:, 0:4 * TILE], packW_d.ap()[:, 0:4 * TILE])
            nc.sync.dma_start(packU[:], packU_d.ap())
            nc.sync.dma_start(packW[:, 4 * TILE:NH * TILE],
                              packW_d.ap()[:, 4 * TILE:NH * TILE])
            nc.sync.dma_start(packW[:, NH * TILE:], packW_d.ap()[:, NH * TILE:])

            # PE warm-up on the pb ring (no readers -> never blocks): keeps
            # the PE p-state ramp alive while input DMAs land and between
            # the early build matmuls, whose PSUM ring waits on evacuation.
            warm_sb = const.tile([TILE, T], bf16, tag="warm")
            nc.vector.memset(warm_sb[:], 0)

            def warm(n=1):
                for _ in range(n):
                    warm_ps = pb.tile([TILE, 2 * T], f32, tag="pb")
                    nc.tensor.matmul(warm_ps[:, 0:T], warm_sb[:, 0:TILE],
                                     warm_sb[:], start=True, stop=True)

            # rcomb: [128, NH*1024], per head h chunk c at h*1024 + c*512,
            # built on the PE: rcomb_{h,c} = packW_{h,c}^T @ U_c, then
            # evacuated PSUM->SBUF by DVE / Scalar.
            rcomb = rhspool.tile([TILE, NH * 2 * T], bf16, tag="rcomb",
                                 name="rcomb")

            def build(h, pre=False):
                for c in range(2):
                    col = 2 * h + c
                    ps = pr.tile([TILE, T], f32, tag="pr")
                    nc.tensor.matmul(
                        ps[:], packW[:, col * TILE:(col + 1) * TILE],
                        packU[:, c * T:(c + 1) * T], start=True, stop=True)
                    if pre and h < 2:
                        warm()
                    dst = rcomb[:, h * 2 * T + c * T: h * 2 * T + (c + 1) * T]
                    if (col % 2 == 1 if pre else col % 10 == 1):
                        nc.scalar.activation(dst, ps[:], Act.Identity)
                    else:
                        nc.vector.tensor_copy(dst, ps[:])

            warm(NWARM)
            for h in range(6):
                build(h, pre=True)

            stages = [stpool.tile([TILE, NH * (i + 1) * TILE], bf16,
                                  tag=f"stage{i}", name=f"stage{i}")
                      for i in range(NT)]
            rc_v = rcomb[:].rearrange("p (h x) -> p h x", h=NH)

            def rslice(h, c, n):
                return rcomb[:, h * 2 * T + c * T: h * 2 * T + c * T + n]

            def evac(dst, src, g, k):
                # last head group: split evacuation across both engines to
                # shorten the tail (DVE is otherwise idle by then)
                if g == 3 and k % 2 == 0:
                    nc.vector.tensor_copy(dst, src)
                else:
                    nc.scalar.activation(dst, src, Act.Identity)

            W2_ = 3 * TILE
            W1_ = 2 * TILE

            def sec_i3(g, h0, nxt):
                # i = 3: head pairs, 2 banks per psum tile, N=512 each
                for m, hp in enumerate((h0, h0 + 2)):
                    ps = pb.tile([TILE, 2 * T], f32, tag="pb")
                    for j in range(2):
                        for c in range(2):
                            nc.tensor.matmul(
                                ps[:, j * T:(j + 1) * T],
                                packU[:, c * T + 3 * TILE: c * T + 4 * TILE],
                                rslice(hp + j, c, T),
                                start=(c == 0), stop=(c == 1),
                                skip_group_check=True)
                    if m == 0 and nxt:
                        build(nxt[0])
                    evac(stages[3][:, hp * T:(hp + 2) * T], ps[:], g, m)

            def sec_i10(g, h0, nxt):
                # i = 1 (two heads per bank) + i = 0 (four heads in one bank;
                # DMA'd straight from PSUM to DRAM as f32 - no evacuation)
                psv = pb.tile([TILE, 2 * T], f32, tag="pb")
                for k, hp in enumerate((h0, h0 + 2)):
                    for c in range(2):
                        nc.tensor.matmul(
                            psv[:, k * T:(k + 1) * T],
                            packU[:, c * T + TILE: c * T + 2 * TILE],
                            rc_v[:, hp:hp + 2, c * T: c * T + W1_],
                            start=(c == 0), stop=(c == 1),
                            skip_group_check=True)
                ps0 = pb.tile([TILE, 2 * T], f32, tag="pb")
                for c in range(2):
                    nc.tensor.matmul(
                        ps0[:, 0:T],
                        packU[:, c * T: c * T + TILE],
                        rc_v[:, h0:h0 + 4, c * T: c * T + TILE],
                        start=(c == 0), stop=(c == 1))
                if len(nxt) > 2:
                    build(nxt[2])
                dst1 = stages[1][:, h0 * W1_:(h0 + 4) * W1_]
                if g == 1:
                    nc.scalar.activation(dst1, psv[:], Act.Identity)
                else:
                    nc.vector.tensor_copy(dst1, psv[:])
                evac(stages[0][:, h0 * TILE:(h0 + 4) * TILE], ps0[:, 0:T], g, 0)
                nc.sync.dma_start(
                    out_ds[0].ap()[:, h0 * TILE:(h0 + 4) * TILE],
                    stages[0][:, h0 * TILE:(h0 + 4) * TILE])

            def sec_i2(g, h0, nxt):
                # i = 2: head pairs, N=384 into each bank of a 2-bank tile
                for m, hp in enumerate((h0, h0 + 2)):
                    ps = pb.tile([TILE, 2 * T], f32, tag="pb")
                    for j in range(2):
                        for c in range(2):
                            nc.tensor.matmul(
                                ps[:, j * T:j * T + W2_],
                                packU[:, c * T + 2 * TILE: c * T + 3 * TILE],
                                rslice(hp + j, c, W2_),
                                start=(c == 0), stop=(c == 1),
                                skip_group_check=True)
                    if m == 0 and len(nxt) > 3:
                        build(nxt[3])
                    evac(stages[2][:, hp * W2_:(hp + 2) * W2_]
                         .rearrange("p (b s) -> p b s", b=2),
                         ps[:].rearrange("p (b s) -> p b s", b=2)[:, :, 0:W2_],
                         g, m)

            def dma_out(i, h0, n=4):
                Wi = (i + 1) * TILE
                nc.sync.dma_start(
                    out_ds[i].ap()[:, h0 * Wi:(h0 + n) * Wi],
                    stages[i][:, h0 * Wi:(h0 + n) * Wi])

            for g in range(4):
                h0 = 4 * g
                nxt = list(range(h0 + 6, min(h0 + 10, NH))) if g < 3 else []
                sec_i3(g, h0, nxt)
                if len(nxt) > 1:
                    build(nxt[1])
                if g < 3:
                    sec_i10(g, h0, nxt)
                    dma_out(3, h0)
                    sec_i2(g, h0, nxt)
                    dma_out(1, h0)
                    dma_out(2, h0)
                else:
                    # last group: big tiles first and finer DMA slices so the
                    # trailing transfer after the final evacuation is short
                    dma_out(3, h0, 2)
                    dma_out(3, h0 + 2, 2)
                    sec_i2(g, h0, nxt)
                    dma_out(2, h0, 2)
                    dma_out(2, h0 + 2, 2)
                    sec_i10(g, h0, nxt)
                    dma_out(1, h0)

    nc.compile()
    return nc


# ---------------------------------------------------------------- host glue
def _to_bf16(x):
    import ml_dtypes
    return np.ascontiguousarray(x, np.float32).astype(ml_dtypes.bfloat16)


def _host_tables(c_sorted, omQ):
    """U: [128, 1024] interleaved cos/sin rows, chunk-major."""
    nQ = len(omQ)
    ang = np.multiply.outer(omQ, c_sorted.astype(np.float64)) * TWO_PI  # [nQ, T]
    cosr = np.cos(ang).astype(np.float32)
    sinr = np.sin(ang).astype(np.float32)
    U = np.zeros((TILE, 2 * T), np.float32)
    for c in range(2):
        for kk in range(64):
            k = 64 * c + kk
            if k >= nQ:
                break
            U[2 * kk, c * T:(c + 1) * T] = cosr[k]
            U[2 * kk + 1, c * T:(c + 1) * T] = sinr[k]
    return U


def _pack_W(a, b, nQ):
    """packW [128, 32*128]: per (h,c) block-diag lhsT so W^T @ U = rcomb.

    rcomb[2j]   = a_j U[2j] - b_j U[2j+1]
    rcomb[2j+1] = b_j U[2j] + a_j U[2j+1]
    """
    a_pad = np.zeros((TILE, NH), np.float64)
    a_pad[:nQ] = a
    b_pad = np.zeros((TILE, NH), np.float64)
    b_pad[1:1 + b.shape[0]] = b
    W = np.zeros((TILE, 2 * NH * TILE), np.float32)
    j = np.arange(64)
    for h in range(NH):
        for c in range(2):
            base = (2 * h + c) * TILE
            k = 64 * c + j
            W[2 * j, base + 2 * j] = a_pad[k, h]
            W[2 * j + 1, base + 2 * j] = -b_pad[k, h]
            W[2 * j, base + 2 * j + 1] = b_pad[k, h]
            W[2 * j + 1, base + 2 * j + 1] = a_pad[k, h]
    return W


def kernel(centers01, mask, bias_phase, W1, b1, W2, b2):
    global _MODULE, _LAST_RESULTS
    from concourse.bass_utils import run_bass_kernel_spmd

    centers01 = np.asarray(centers01, np.float32)
    bias_phase = np.asarray(bias_phase, np.float64)
    W1 = np.asarray(W1, np.float64)
    b1 = np.asarray(b1, np.float64)
    W2 = np.asarray(W2, np.float64)
    b2 = np.asarray(b2, np.float64)

    ck = hash((bias_phase.tobytes(), W1.tobytes(), b1.tobytes(),
               W2.tobytes(), b2.tobytes()))
    if ck not in _FIT_CACHE:
        _FIT_CACHE[ck] = _fit_coefs(bias_phase, W1, b1, W2, b2)
    omQ, omP, a, b, _gridmax = _FIT_CACHE[ck]
    nQ = len(omQ)

    packW = _to_bf16(_pack_W(a, b, nQ))

    if _MODULE is None:
        _MODULE = _build_module()
    nc = _MODULE

    in_maps = []
    idxs = []
    for bi in range(N_CORES):
        c = centers01[bi]
        idx = np.argsort(c, kind="stable")
        idxs.append(idx)
        U = _host_tables(c[idx], omQ)
        in_maps.append({
            "packU": _to_bf16(U),
            "packW": packW,
        })

    res = run_bass_kernel_spmd(nc, in_maps, list(range(N_CORES)))
    _LAST_RESULTS = res

    out = np.empty((B, NH, T, T), np.float32)
    M = np.empty((NH, T, T), np.float32)
    iu = np.triu_indices(T, 1)
    for bi in range(N_CORES):
        for i in range(NT):
            Wi = (i + 1) * TILE
            raw = np.asarray(res.results[bi][f"out{i}"])
            if raw.dtype != np.uint16:
                raw = raw.view(np.uint16)
            f = (raw.astype(np.uint32) << 16).view(np.float32)
            M[:, i * TILE:(i + 1) * TILE, 0:Wi] = \
                f.reshape(TILE, NH, Wi).transpose(1, 0, 2)
        M[:, iu[0], iu[1]] = M[:, iu[1], iu[0]]
        inv = np.empty(T, np.int64)
        inv[idxs[bi]] = np.arange(T)
        out[bi] = M[:, inv][:, :, inv]
    m = np.asarray(mask, bool)
    if not m.all():
        out *= (m[:, None, :, None] & m[:, None, None, :]).astype(np.float32)
    return out


# revision 35
# speedup vs baseline: 1.1126x; 1.0115x over previous
"""Trainium2 Bass kernel for nn_RelPosRFFBias — factorized Fourier, v5 (full host mirror).

Math: per head h, bias(t,s) = g_h(|c_t - c_s|) with g_h fit as a ~125-tone
cosine+sine series.  In sorted-center order, for t >= s (lower triangle):

  g(d) = sum_k a_k cos(w_k d) + b_k sin(w_k d)
       = sum_k cos_t (a_k cos_s - b_k sin_s) + sin_t (a_k sin_s + b_k cos_s)

so ONE rank-256 matmul per (row-tile, head) with lhs = raw interleaved
cos/sin table U and rhs = rcomb = aq (.) U + bq (.) V (V = pair-swapped U)
evaluates the whole lower block-triangle, diagonal tiles included.  The host
mirrors the strict upper triangle (inter- and intra-tile) by symmetry and
undoes the sort permutation.

v5 vs v4 baseline:
 - no separate P matmuls / diag sign fixups: PE work halves (43us -> ~18us).
 - builds are 2 DVE passes per (head, chunk): tensor_scalar + fused
   scalar_tensor_tensor, both 4x-mode eligible.
 - PSUM evacuation split across Scalar (i=3, i=1) and GpSimd (i=2, i=0).
 - warmup trimmed to ~7 matmuls (p-state ramp needs ~3us, not 6.2us).
 - head-group-column-major schedule: output DMA per (row-tile, 4-head group)
   spread across SP/Act/DVE queues; short tail.
"""

import math

import numpy as np

B, T = 8, 512
RFF, NH = 16, 16
F_MIN, F_MAX = 2.0, 64.0
TWO_PI = 2.0 * math.pi

N_CORES = 8
L_PER = 1.0625
KU = 124
NQMAX = 128
TILE = 128
NT = T // TILE
FIT_LAM = 1e-5
FIT_ITERS = 14
NWARM = 5

_MODULE = None
_LAST_RESULTS = None
_FIT_CACHE = {}


# ---------------------------------------------------------------- host: fit
def _gelu64(x):
    try:
        from scipy.special import erf
    except ImportError:
        erf = np.vectorize(math.erf)
    return 0.5 * x * (1.0 + erf(x / math.sqrt(2.0)))


def _g_of_D(D, phase, W1, b1, W2, b2, freqs):
    arg = TWO_PI * D[:, None] * freqs[None, :] + phase[None, :]
    feats = np.concatenate([np.sin(arg), np.cos(arg)], axis=-1)
    return _gelu64(feats @ W1 + b1) @ W2 + b2


def _tone_grid():
    freqs = np.logspace(math.log10(F_MIN), math.log10(F_MAX), RFF).astype(np.float64)
    uni = np.arange(KU) / L_PER
    cut = uni[-1]
    cand = sorted(set(
        round(f, 6)
        for f in np.concatenate([(freqs[:, None] + freqs[None, :]).ravel(), 2 * freqs])
        if cut + 0.2 < f < 145.0
    ))
    omQ = np.concatenate([uni, np.asarray(cand[: NQMAX - KU], dtype=np.float64)])
    omQ.sort()
    omP = omQ[1:min(len(omQ), 129)]
    return freqs, omQ, omP


def _fit_coefs(phase, W1, b1, W2, b2):
    freqs, omQ, omP = _tone_grid()
    NG = 32768
    Dg = (np.arange(NG) + 0.5) / NG
    G = _g_of_D(Dg, phase, W1, b1, W2, b2, freqs)
    Phi = np.concatenate(
        [np.cos(Dg[:, None] * TWO_PI * omQ[None, :]),
         np.sin(Dg[:, None] * TWO_PI * omP[None, :])], axis=1)
    lam = FIT_LAM * NG
    w = np.ones(NG)
    best = None
    for _ in range(FIT_ITERS):
        Pw = Phi * w[:, None]
        A = Pw.T @ Phi
        A[np.diag_indices_from(A)] += lam
        coef = np.linalg.solve(A, Pw.T @ G)
        res = np.abs(Phi @ coef - G).max(axis=1)
        mx = res.max()
        if best is None or mx < best[0]:
            best = (mx, coef.copy())
        w = w * (0.05 + res / mx)
        w = np.maximum(w / w.mean(), 1e-6)
    mx, coef = best
    return omQ, omP, coef[: len(omQ)], coef[len(omQ):], mx


# ---------------------------------------------------------------- device
def _build_module():
    import concourse.tile as tile
    from concourse import bacc, mybir
    from contextlib import ExitStack

    f32 = mybir.dt.float32
    bf16 = mybir.dt.bfloat16
    Alu = mybir.AluOpType
    Act = mybir.ActivationFunctionType

    nc = bacc.Bacc("TRN2", target_bir_lowering=False, debug=False)

    # packU: raw interleaved cos/sin table, chunk-major [U0 | U1]
    packU_d = nc.dram_tensor("packU", [TILE, 2 * T], bf16, kind="ExternalInput")
    # packW: per (h, c) 2x2-block-diagonal rotation/scale lhsT [128,128],
    # col block at (2h+c)*128.  rcomb_{h,c} = packW_{h,c}^T @ U_c on the PE.
    packW_d = nc.dram_tensor("packW", [TILE, 2 * NH * TILE], bf16,
                             kind="ExternalInput")
    # compact block-lower-triangle outputs, head-major per row-tile i
    out_ds = [nc.dram_tensor(f"out{i}", [TILE, NH * (i + 1) * TILE], bf16,
                             kind="ExternalOutput") for i in range(NT)]

    with tile.TileContext(nc) as tc:
        with ExitStack() as ctx:
            const = ctx.enter_context(tc.tile_pool(name="const", bufs=1))
            rhspool = ctx.enter_context(tc.tile_pool(name="rhs", bufs=1))
            stpool = ctx.enter_context(tc.tile_pool(name="stage", bufs=1))
            # PSUM: pb = [128,1024] (2 banks) x3 bufs for i3/i2 head pairs,
            # the i1 pair-of-pairs, and warmups; pr = [128,512] x2 bufs for
            # rcomb builds + i0.  6 + 2 = 8 banks.
            pb = ctx.enter_context(tc.tile_pool(name="pb", bufs=3, space="PSUM"))
            pr = ctx.enter_context(tc.tile_pool(name="pr", bufs=2, space="PSUM"))

            packW = const.tile([TILE, 2 * NH * TILE], bf16, tag="packW")
            packU = const.tile([TILE, 2 * T], bf16, tag="packU")
            # first bytes on the critical path: W blocks for h0/h1, then U,
            # then the rest of W
            nc.sync.dma_start(packW[:, 0:4 * TILE], packW_d.ap()[:, 0:4 * TILE])
            nc.sync.dma_start(packU[:], packU_d.ap())
            nc.sync.dma_start(packW[:, 4 * TILE:NH * TILE],
                              packW_d.ap()[:, 4 * TILE:NH * TILE])
            nc.sync.dma_start(packW[:, NH * TILE:], packW_d.ap()[:, NH * TILE:])

            # PE warm-up on the pb ring (no readers -> never blocks): keeps
            # the PE p-state ramp alive while input DMAs land and between
            # the early build matmuls, whose PSUM ring waits on evacuation.
            warm_sb = const.tile([TILE, T], bf16, tag="warm")
            nc.gpsimd.memset(warm_sb[:], 0)

            def warm(n=1):
                for _ in range(n):
                    warm_ps = pb.tile([TILE, 2 * T], f32, tag="pb")
                    nc.tensor.matmul(warm_ps[:, 0:T], warm_sb[:, 0:TILE],
                                     warm_sb[:], start=True, stop=True)

            # rcomb: [128, NH*1024], per head h chunk c at h*1024 + c*512,
            # built on the PE: rcomb_{h,c} = packW_{h,c}^T @ U_c, then
            # evacuated PSUM->SBUF by DVE / Scalar.
            rcomb = rhspool.tile([TILE, NH * 2 * T], bf16, tag="rcomb",
                                 name="rcomb")

            def build(h, pre=False):
                for c in range(2):
                    col = 2 * h + c
                    ps = pr.tile([TILE, T], f32, tag="pr")
                    nc.tensor.matmul(
                        ps[:], packW[:, col * TILE:(col + 1) * TILE],
                        packU[:, c * T:(c + 1) * T], start=True, stop=True)
                    if pre and h < 2:
                        warm()
                    dst = rcomb[:, h * 2 * T + c * T: h * 2 * T + (c + 1) * T]
                    if (col % 2 == 1 if pre else col % 10 == 1):
                        nc.scalar.activation(dst, ps[:], Act.Identity)
                    else:
                        nc.vector.tensor_copy(dst, ps[:])

            warm(NWARM)
            for h in range(4):
                build(h, pre=True)

            stages = [stpool.tile([TILE, NH * (i + 1) * TILE], bf16,
                                  tag=f"stage{i}", name=f"stage{i}")
                      for i in range(NT)]
            rc_v = rcomb[:].rearrange("p (h x) -> p h x", h=NH)

            def rslice(h, c, n):
                return rcomb[:, h * 2 * T + c * T: h * 2 * T + c * T + n]

            def evac(dst, src, g, k):
                # last head group: split evacuation across both engines to
                # shorten the tail (DVE is otherwise idle by then)
                if g == 3 and k % 2 == 0:
                    nc.vector.tensor_copy(dst, src)
                else:
                    nc.scalar.activation(dst, src, Act.Identity)

            W2_ = 3 * TILE
            W1_ = 2 * TILE

            def sec_i3(g, h0, nxt):
                # i = 3: head pairs, 2 banks per psum tile, N=512 each
                for m, hp in enumerate((h0, h0 + 2)):
                    ps = pb.tile([TILE, 2 * T], f32, tag="pb")
                    for j in range(2):
                        for c in range(2):
                            nc.tensor.matmul(
                                ps[:, j * T:(j + 1) * T],
                                packU[:, c * T + 3 * TILE: c * T + 4 * TILE],
                                rslice(hp + j, c, T),
                                start=(c == 0), stop=(c == 1),
                                skip_group_check=True)
                    if m == 0 and nxt:
                        build(nxt[0])
                    evac(stages[3][:, hp * T:(hp + 2) * T], ps[:], g, m)

            def sec_i10(g, h0, nxt):
                # i = 1 (two heads per bank) + i = 0 (four heads in one bank;
                # DMA'd straight from PSUM to DRAM as f32 - no evacuation)
                psv = pb.tile([TILE, 2 * T], f32, tag="pb")
                for k, hp in enumerate((h0, h0 + 2)):
                    for c in range(2):
                        nc.tensor.matmul(
                            psv[:, k * T:(k + 1) * T],
                            packU[:, c * T + TILE: c * T + 2 * TILE],
                            rc_v[:, hp:hp + 2, c * T: c * T + W1_],
                            start=(c == 0), stop=(c == 1),
                            skip_group_check=True)
                ps0 = pb.tile([TILE, 2 * T], f32, tag="pb")
                for c in range(2):
                    nc.tensor.matmul(
                        ps0[:, 0:T],
                        packU[:, c * T: c * T + TILE],
                        rc_v[:, h0:h0 + 4, c * T: c * T + TILE],
                        start=(c == 0), stop=(c == 1))
                if len(nxt) > 2:
                    build(nxt[2])
                dst1 = stages[1][:, h0 * W1_:(h0 + 4) * W1_]
                if g in (1, 3):
                    nc.scalar.activation(dst1, psv[:], Act.Identity)
                else:
                    nc.vector.tensor_copy(dst1, psv[:])
                evac(stages[0][:, h0 * TILE:(h0 + 4) * TILE], ps0[:, 0:T], g, 0)
                nc.sync.dma_start(
                    out_ds[0].ap()[:, h0 * TILE:(h0 + 4) * TILE],
                    stages[0][:, h0 * TILE:(h0 + 4) * TILE])

            def sec_i2(g, h0, nxt):
                # i = 2: head pairs, N=384 into each bank of a 2-bank tile
                for m, hp in enumerate((h0, h0 + 2)):
                    ps = pb.tile([TILE, 2 * T], f32, tag="pb")
                    for j in range(2):
                        for c in range(2):
                            nc.tensor.matmul(
                                ps[:, j * T:j * T + W2_],
                                packU[:, c * T + 2 * TILE: c * T + 3 * TILE],
                                rslice(hp + j, c, W2_),
                                start=(c == 0), stop=(c == 1),
                                skip_group_check=True)
                    if m == 0 and len(nxt) > 3:
                        build(nxt[3])
                    evac(stages[2][:, hp * W2_:(hp + 2) * W2_]
                         .rearrange("p (b s) -> p b s", b=2),
                         ps[:].rearrange("p (b s) -> p b s", b=2)[:, :, 0:W2_],
                         g, m)

            def dma_out(i, h0, n=4):
                Wi = (i + 1) * TILE
                nc.sync.dma_start(
                    out_ds[i].ap()[:, h0 * Wi:(h0 + n) * Wi],
                    stages[i][:, h0 * Wi:(h0 + n) * Wi])

            built = 4
            for g in range(4):
                h0 = 4 * g
                n_new = min(6 if g == 0 else 4, NH - built)
                nxt = list(range(built, built + n_new))
                built += n_new
                sec_i3(g, h0, nxt)
                if len(nxt) > 1:
                    build(nxt[1])
                if g < 3:
                    sec_i10(g, h0, nxt)
                    dma_out(3, h0)
                    sec_i2(g, h0, nxt)
                    if len(nxt) > 4:
                        build(nxt[4])
                        build(nxt[5])
                    dma_out(1, h0)
                    dma_out(2, h0)
                else:
                    # last group: big tiles first and finer DMA slices so the
                    # trailing transfer after the final evacuation is short
                    dma_out(3, h0, 2)
                    dma_out(3, h0 + 2, 2)
                    sec_i2(g, h0, nxt)
                    dma_out(2, h0, 2)
                    dma_out(2, h0 + 2, 2)
                    sec_i10(g, h0, nxt)
                    dma_out(1, h0)

    nc.compile()
    return nc


# ---------------------------------------------------------------- host glue
def _to_bf16(x):
    import ml_dtypes
    return np.ascontiguousarray(x, np.float32).astype(ml_dtypes.bfloat16)


def _host_tables(c_sorted, omQ):
    """U: [128, 1024] interleaved cos/sin rows, chunk-major."""
    nQ = len(omQ)
    ang = np.multiply.outer(omQ, c_sorted.astype(np.float64)) * TWO_PI  # [nQ, T]
    cosr = np.cos(ang).astype(np.float32)
    sinr = np.sin(ang).astype(np.float32)
    U = np.zeros((TILE, 2 * T), np.float32)
    for c in range(2):
        for kk in range(64):
            k = 64 * c + kk
            if k >= nQ:
                break
            U[2 * kk, c * T:(c + 1) * T] = cosr[k]
            U[2 * kk + 1, c * T:(c + 1) * T] = sinr[k]
    return U


def _pack_W(a, b, nQ):
    """packW [128, 32*128]: per (h,c) block-diag lhsT so W^T @ U = rcomb.

    rcomb[2j]   = a_j U[2j] - b_j U[2j+1]
    rcomb[2j+1] = b_j U[2j] + a_j U[2j+1]
    """
    a_pad = np.zeros((TILE, NH), np.float64)
    a_pad[:nQ] = a
    b_pad = np.zeros((TILE, NH), np.float64)
    b_pad[1:1 + b.shape[0]] = b
    W = np.zeros((TILE, 2 * NH * TILE), np.float32)
    j = np.arange(64)
    for h in range(NH):
        for c in range(2):
            base = (2 * h + c) * TILE
            k = 64 * c + j
            W[2 * j, base + 2 * j] = a_pad[k, h]
            W[2 * j + 1, base + 2 * j] = -b_pad[k, h]
            W[2 * j, base + 2 * j + 1] = b_pad[k, h]
            W[2 * j + 1, base + 2 * j + 1] = a_pad[k, h]
    return W


def kernel(centers01, mask, bias_phase, W1, b1, W2, b2):
    global _MODULE, _LAST_RESULTS
    from concourse.bass_utils import run_bass_kernel_spmd

    centers01 = np.asarray(centers01, np.float32)
    bias_phase = np.asarray(bias_phase, np.float64)
    W1 = np.asarray(W1, np.float64)
    b1 = np.asarray(b1, np.float64)
    W2 = np.asarray(W2, np.float64)
    b2 = np.asarray(b2, np.float64)

    ck = hash((bias_phase.tobytes(), W1.tobytes(), b1.tobytes(),
               W2.tobytes(), b2.tobytes()))
    if ck not in _FIT_CACHE:
        _FIT_CACHE[ck] = _fit_coefs(bias_phase, W1, b1, W2, b2)
    omQ, omP, a, b, _gridmax = _FIT_CACHE[ck]
    nQ = len(omQ)

    packW = _to_bf16(_pack_W(a, b, nQ))

    if _MODULE is None:
        _MODULE = _build_module()
    nc = _MODULE

    in_maps = []
    idxs = []
    for bi in range(N_CORES):
        c = centers01[bi]
        idx = np.argsort(c, kind="stable")
        idxs.append(idx)
        U = _host_tables(c[idx], omQ)
        in_maps.append({
            "packU": _to_bf16(U),
            "packW": packW,
        })

    res = run_bass_kernel_spmd(nc, in_maps, list(range(N_CORES)))
    _LAST_RESULTS = res

    out = np.empty((B, NH, T, T), np.float32)
    M = np.empty((NH, T, T), np.float32)
    iu = np.triu_indices(T, 1)
    for bi in range(N_CORES):
        for i in range(NT):
            Wi = (i + 1) * TILE
            raw = np.asarray(res.results[bi][f"out{i}"])
            if raw.dtype != np.uint16:
                raw = raw.view(np.uint16)
            f = (raw.astype(np.uint32) << 16).view(np.float32)
            M[:, i * TILE:(i + 1) * TILE, 0:Wi] = \
                f.reshape(TILE, NH, Wi).transpose(1, 0, 2)
        M[:, iu[0], iu[1]] = M[:, iu[1], iu[0]]
        inv = np.empty(T, np.int64)
        inv[idxs[bi]] = np.arange(T)
        out[bi] = M[:, inv][:, :, inv]
    m = np.asarray(mask, bool)
    if not m.all():
        out *= (m[:, None, :, None] & m[:, None, None, :]).astype(np.float32)
    return out
